# revision 3
# baseline (speedup 1.0000x reference)
"""Trainium2 Bass kernel for an LSTM-style block (RMSNorm -> gates -> linear
recurrence scan -> output projection + residual).

Sharding: 8 cores = data-parallel over batch (4) x tensor-parallel over
proj_dim halves (2).  Each core processes one batch element and 682 of the
1364 channels (padded to 768 for 128-alignment).  The recurrence is
elementwise per channel, so no cross-core communication is needed; the
output projection produces per-core partial sums over the channel half,
each carrying half the residual, and the host adds the two halves.

Device layout per core (time chunked into 16 x 256 tokens):
  - RMSNorm on [128 tok, 1024] tiles, cast to bf16, PE-transpose to
    xnT [d, time] so gate matmuls produce [channel, time] directly.
  - Gate/cell matmuls in bf16, K=1024 accumulated in PSUM.
  - softcap sigmoid(15*tanh(g/15)) via two ACT passes; tanh for the cell.
  - h_t = f_t*h_{t-1} + i_t*tanh(c_t) via the DVE tensor_tensor_scan
    instruction (one scan per 128-channel block, carried across chunks).
  - out = o*tanh(h) (bf16) -> matmul with W_out -> +0.5*x residual -> DRAM.
"""

import numpy as np
import ml_dtypes

import concourse.bass as bass  # noqa: F401  (bass types used via bacc)
import concourse.tile as tile
import concourse.mybir as mybir
from concourse import bacc
from concourse.bass_utils import run_bass_kernel_spmd
from concourse.masks import make_identity

B, S, D = 4, 4096, 1024
PF = 1364          # proj dim (int(1024*1.333))
H = PF // 2        # 682 channels per core
HP = 768           # padded per-core channels (6 x 128)
CAP = 15.0
EPS = 1e-6
NKB = D // 128     # 8 k-blocks over D
NCB = HP // 128    # 6 channel blocks per core
CH = 256           # time chunk
NCH = S // CH      # 16 chunks
NTT = CH // 128    # 2 token tiles per chunk
NH = D // 512      # 2 halves of the output dim per PSUM bank

F32 = mybir.dt.float32
BF16 = mybir.dt.bfloat16

_NC_CACHE = {}


def _build_nc(s_len=S):
    if s_len in _NC_CACHE:
        return _NC_CACHE[s_len]
    nch = s_len // CH

    mult = mybir.AluOpType.mult
    add = mybir.AluOpType.add
    ACT = mybir.ActivationFunctionType

    nc = bacc.Bacc("TRN2", target_bir_lowering=False, debug=False, num_devices=8)
    x_d = nc.dram_tensor("x", [s_len, D], F32, kind="ExternalInput")
    wg_d = nc.dram_tensor("wg", [D, 3 * HP], BF16, kind="ExternalInput")
    wc_d = nc.dram_tensor("wc", [D, HP], BF16, kind="ExternalInput")
    wo_d = nc.dram_tensor("wo", [HP, D], BF16, kind="ExternalInput")
    h0_d = nc.dram_tensor("h0", [128, NCB], F32, kind="ExternalInput")
    y_d = nc.dram_tensor("y", [s_len, D], F32, kind="ExternalOutput")

    with tile.TileContext(nc) as tc:
        with (
            tc.tile_pool(name="wpool", bufs=1) as wpool,
            tc.tile_pool(name="xpool", bufs=4) as xpool,
            tc.tile_pool(name="xnbp", bufs=2) as xnbp,
            tc.tile_pool(name="smallp", bufs=4) as smallp,
            tc.tile_pool(name="xntp", bufs=2) as xntp,
            tc.tile_pool(name="gpool", bufs=1) as gpool,
            tc.tile_pool(name="ttmpp", bufs=4) as ttmpp,
            tc.tile_pool(name="scanp", bufs=1) as scanp,
            tc.tile_pool(name="hpool", bufs=2) as hpool,
            tc.tile_pool(name="obfp", bufs=2) as obfp,
            tc.tile_pool(name="outp", bufs=3) as outp,
            tc.tile_pool(name="psg", bufs=4, space="PSUM") as psg,
            tc.tile_pool(name="pst", bufs=2, space="PSUM") as pst,
            tc.tile_pool(name="pso", bufs=2, space="PSUM") as pso,
        ):
            # ---- resident tensors ----
            wg_s = wpool.tile([128, NKB, 3 * HP], BF16)
            nc.sync.dma_start(out=wg_s, in_=wg_d.rearrange("(kb p) c -> p kb c", p=128))
            wc_s = wpool.tile([128, NKB, HP], BF16)
            nc.sync.dma_start(out=wc_s, in_=wc_d.rearrange("(kb p) c -> p kb c", p=128))
            wo_s = wpool.tile([128, NCB, D], BF16)
            nc.sync.dma_start(out=wo_s, in_=wo_d.rearrange("(cb p) d -> p cb d", p=128))
            h0_s = wpool.tile([128, NCB], F32)
            nc.sync.dma_start(out=h0_s, in_=h0_d[:, :])
            ident = wpool.tile([128, 128], BF16)
            make_identity(nc, ident)
            eps_t = wpool.tile([128, 1], F32)
            nc.vector.memset(eps_t, EPS)

            h_prev = None
            for ci in range(nch):
                t0 = ci * CH
                # ---- phase A: rmsnorm + transpose ----
                xa_tiles = []
                xnT = xntp.tile([128, NKB, CH], BF16, tag="xnT", name="xnT")
                for tt in range(NTT):
                    r0 = t0 + tt * 128
                    xa = xpool.tile([128, D], F32, tag="xa", name="xa")
                    nc.sync.dma_start(out=xa, in_=x_d[r0:r0 + 128, :])
                    xa_tiles.append(xa)
                    xnb = xnbp.tile([128, D], BF16, tag="xnb", name="xnb")
                    ssq = smallp.tile([128, 1], F32, tag="ssq", name="ssq")
                    # xnb <- x*x (scratch, overwritten below), ssq <- sum(x*x)
                    nc.vector.tensor_mul(out=xnb, in0=xa, in1=xa)
                    nc.vector.tensor_reduce(out=ssq, in_=xnb,
                                            axis=mybir.AxisListType.X, op=add)
                    rs = smallp.tile([128, 1], F32, tag="rs", name="rs")
                    nc.scalar.activation(out=rs, in_=ssq, func=ACT.Sqrt,
                                         bias=eps_t, scale=1.0 / D)
                    nc.vector.reciprocal(out=rs, in_=rs)
                    # xnb <- x * rsqrt(mean(x^2)+eps), cast bf16
                    nc.scalar.activation(out=xnb, in_=xa, func=ACT.Copy, scale=rs)
                    for kb in range(NKB):
                        ptr = pst.tile([128, 128], BF16, tag="ptr", name="ptr")
                        nc.tensor.transpose(ptr, xnb[:, kb * 128:(kb + 1) * 128], ident)
                        nc.vector.tensor_copy(
                            out=xnT[:, kb, tt * 128:(tt + 1) * 128], in_=ptr)

                # ---- phase B: gate + cell matmuls, activations ----
                iact = gpool.tile([128, NCB, CH], F32, tag="iact", name="iact")
                fact = gpool.tile([128, NCB, CH], F32, tag="fact", name="fact")
                oact = gpool.tile([128, NCB, CH], F32, tag="oact", name="oact")
                tca = gpool.tile([128, NCB, CH], F32, tag="tca", name="tca")
                for gi, gt in enumerate((iact, fact, oact)):
                    for cb in range(NCB):
                        pg = psg.tile([128, CH], F32, tag="pg", name="pg")
                        m0 = gi * HP + cb * 128
                        for kb in range(NKB):
                            nc.tensor.matmul(
                                pg, wg_s[:, kb, m0:m0 + 128], xnT[:, kb, :],
                                start=(kb == 0), stop=(kb == NKB - 1))
                        tmp = ttmpp.tile([128, CH], F32, tag="ttmp", name="tmp")
                        nc.scalar.activation(out=tmp, in_=pg, func=ACT.Tanh,
                                             scale=1.0 / CAP)
                        nc.scalar.activation(out=gt[:, cb, :], in_=tmp,
                                             func=ACT.Sigmoid, scale=CAP)
                for cb in range(NCB):
                    pc = psg.tile([128, CH], F32, tag="pg", name="pc")
                    c0 = cb * 128
                    for kb in range(NKB):
                        nc.tensor.matmul(
                            pc, wc_s[:, kb, c0:c0 + 128], xnT[:, kb, :],
                            start=(kb == 0), stop=(kb == NKB - 1))
                    nc.scalar.activation(out=tca[:, cb, :], in_=pc, func=ACT.Tanh)

                # ---- phase C: scan + output gating ----
                bsc = scanp.tile([128, NCB, CH], F32, tag="bsc", name="bsc")
                nc.vector.tensor_mul(
                    out=bsc.rearrange("p a b -> p (a b)"),
                    in0=iact.rearrange("p a b -> p (a b)"),
                    in1=tca.rearrange("p a b -> p (a b)"))
                h_t = hpool.tile([128, NCB, CH], F32, tag="h", name="h_t")
                for cb in range(NCB):
                    init = (h0_s[:, cb:cb + 1] if ci == 0
                            else h_prev[:, cb, CH - 1:CH])
                    nc.vector.tensor_tensor_scan(
                        out=h_t[:, cb, :], data0=fact[:, cb, :],
                        data1=bsc[:, cb, :], initial=init, op0=mult, op1=add)
                th = scanp.tile([128, NCB, CH], F32, tag="th", name="th")
                nc.scalar.activation(out=th.rearrange("p a b -> p (a b)"),
                                     in_=h_t.rearrange("p a b -> p (a b)"),
                                     func=ACT.Tanh)
                obf = obfp.tile([128, NCB, CH], BF16, tag="obf", name="obf")
                nc.vector.tensor_mul(
                    out=obf.rearrange("p a b -> p (a b)"),
                    in0=oact.rearrange("p a b -> p (a b)"),
                    in1=th.rearrange("p a b -> p (a b)"))
                h_prev = h_t

                # ---- phase D: output projection + residual ----
                for tt in range(NTT):
                    r0 = t0 + tt * 128
                    osb = outp.tile([128, D], F32, tag="osb", name="osb")
                    for nh in range(NH):
                        po = pso.tile([128, 512], F32, tag="po", name="po")
                        for cb in range(NCB):
                            nc.tensor.matmul(
                                po, obf[:, cb, tt * 128:(tt + 1) * 128],
                                wo_s[:, cb, nh * 512:(nh + 1) * 512],
                                start=(cb == 0), stop=(cb == NCB - 1))
                        # osb = 0.5*x + proj  (each core carries half the residual)
                        nc.vector.scalar_tensor_tensor(
                            out=osb[:, nh * 512:(nh + 1) * 512],
                            in0=xa_tiles[tt][:, nh * 512:(nh + 1) * 512],
                            scalar=0.5, in1=po, op0=mult, op1=add)
                    nc.sync.dma_start(out=y_d[r0:r0 + 128, :], in_=osb)

    nc.compile()
    _NC_CACHE[s_len] = nc
    return nc


def _prep_inputs(x, hidden_state, ln_weight, W_gate, W_cell, W_out):
    """Host-side prep: fold ln_weight into the weights, slice per-core
    channel halves, pad 682 -> 768 with zeros, cast weights to bf16."""
    x = np.asarray(x, np.float32)
    hidden_state = np.asarray(hidden_state, np.float32)
    ln = np.asarray(ln_weight, np.float32)
    Wg = np.asarray(W_gate, np.float32).reshape(D, 3, PF) * ln[:, None, None]
    Wc = np.asarray(W_cell, np.float32) * ln[:, None]
    Wo = np.asarray(W_out, np.float32)

    in_maps = []
    for c in range(8):
        b, hf = c // 2, c % 2
        cols = slice(hf * H, (hf + 1) * H)
        wg_l = np.zeros((D, 3, HP), np.float32)
        wg_l[:, :, :H] = Wg[:, :, cols]
        wc_l = np.zeros((D, HP), np.float32)
        wc_l[:, :H] = Wc[:, cols]
        wo_l = np.zeros((HP, D), np.float32)
        wo_l[:H, :] = Wo[cols, :]
        h0_l = np.zeros((HP,), np.float32)
        h0_l[:H] = hidden_state[b, cols]
        in_maps.append({
            "x": np.ascontiguousarray(x[b]),
            "wg": wg_l.reshape(D, 3 * HP).astype(ml_dtypes.bfloat16),
            "wc": wc_l.astype(ml_dtypes.bfloat16),
            "wo": wo_l.astype(ml_dtypes.bfloat16),
            "h0": np.ascontiguousarray(h0_l.reshape(NCB, 128).T),
        })
    return in_maps


def _run(inputs, trace=False):
    s_len = inputs["x"].shape[1]
    nc = _build_nc(s_len)
    in_maps = _prep_inputs(**inputs)
    res = run_bass_kernel_spmd(nc, in_maps, core_ids=list(range(8)), trace=trace)
    out = np.empty((B, s_len, D), np.float32)
    for b in range(B):
        out[b] = res.results[2 * b]["y"] + res.results[2 * b + 1]["y"]
    return out, res


def kernel(x, hidden_state, ln_weight, W_gate, W_cell, W_out):
    out, _ = _run(dict(x=x, hidden_state=hidden_state, ln_weight=ln_weight,
                       W_gate=W_gate, W_cell=W_cell, W_out=W_out))
    # the module returns the UNMODIFIED initial hidden_state
    return out, np.asarray(hidden_state, np.float32)
